# revision 3
# baseline (speedup 1.0000x reference)
"""DenseGCN Trainium2 kernel (8-core SPMD).

Strategy (1D node partitioning, edge-cut by dst):
- Nodes are range-sharded: core c owns nodes [c*NPC, (c+1)*NPC).
- Edges live on the core that owns their dst. Within a core, edges are
  bucketed by 128-node dst group and padded so every group has exactly
  K tiles of 128 edges (K derived from the data at build time).
- Per edge tile: dma_gather pulls node-PAIR rows (512 B) from a
  DRAM-replicated h table ([N/2, 2H] view, int16 pair index = src>>1),
  ScalarE scales the lo/hi half by ew*(1-src&1) / ew*(src&1), DVE builds
  a one-hot dst matrix C via is_equal(iota, local_dst), and two TensorE
  matmuls (lhsT=C, rhs=scaled lo/hi halves) accumulate the segment sum
  for the dst group directly in PSUM.  No scatter DMA anywhere.
- Group epilogue: PSUM agg -> transpose -> conv matmul -> +bias -> LN ->
  residual bookkeeping, all on-chip into an SBUF-resident h shard.
- Between conv layers an AllGather replicates the 2 MB h shards into the
  16 MB gather table.
"""

import math

import numpy as np

import concourse.bacc as bacc
import concourse.bass as bass
import concourse.mybir as mybir
import concourse.tile as tile
from concourse import library_config
from concourse.bass_utils import run_bass_kernel_spmd

F32 = mybir.dt.float32
I16 = mybir.dt.int16
I32 = mybir.dt.int32
AF = mybir.ActivationFunctionType
ALU = mybir.AluOpType
AX = mybir.AxisListType

NC_CORES = 8
F_IN = 128
H = 64
OUT = 32
L = 3
EPS = 1e-5
GROUP = 128  # dst nodes per segment-sum group (= PE output partitions)
P = 128

# debug bisect switches: "convs" (skip conv layers), "gather" (memset
# instead of dma_gather), "collective" (plain DMA instead of AllGather)
DEBUG_SKIP = set()


def _wrap_tile_major(v, T):
    """[T*128] -> [128, T] with v[t*128+p] at [p, t]."""
    return np.ascontiguousarray(v.reshape(T, P).T)


def _wrap_idx16(idx, E_s):
    """[E_s] int -> [128, E_s/16] int16, 16-partition wrap replicated 8x."""
    assert E_s % 16 == 0
    w16 = idx.reshape(E_s // 16, 16).T.astype(np.int16)  # [16, E_s/16]
    return np.ascontiguousarray(np.tile(w16, (8, 1)))  # [128, E_s/16]


def prep_inputs(x, edge_weight, src, dst, n_nodes, npc):
    """Host-side shard + edge bucketing.

    Edges go to the core owning dst, split into lo (src < N/2) and hi
    streams, bucketed by 128-node dst group, each (core, group, half)
    run padded to whole 128-edge tiles; all cores padded to the same
    K_lo / K_hi tiles per group.  Returns per-core dicts + (K_lo, K_hi).
    """
    ew = edge_weight.reshape(-1).astype(np.float32)
    src = src.astype(np.int64)
    dst = dst.astype(np.int64)
    ngroups = npc // GROUP
    half = n_nodes // 2

    per_core = []
    kmax = [1, 1]
    for c in range(NC_CORES):
        m = (dst // npc) == c
        s_c, d_c, w_c = src[m], dst[m], ew[m]
        halves = []
        for h in range(2):
            hm = (s_c >= half) == bool(h)
            s_h, d_h, w_h = s_c[hm], d_c[hm], w_c[hm]
            g = (d_h % npc) // GROUP
            order = np.argsort(g, kind="stable")
            s_h, d_h, w_h, g = s_h[order], d_h[order], w_h[order], g[order]
            cnt = np.bincount(g, minlength=ngroups)
            kmax[h] = max(kmax[h], math.ceil(int(cnt.max()) / P))
            halves.append((s_h, d_h, w_h, g, cnt))
        per_core.append(halves)

    K_lo, K_hi = kmax

    maps = []
    for c in range(NC_CORES):
        out = {
            "x": np.ascontiguousarray(x[c * npc : (c + 1) * npc]).astype(
                np.float32
            )
        }
        for h, K in ((0, K_lo), (1, K_hi)):
            s_h, d_h, w_h, g, cnt = per_core[c][h]
            E_s = ngroups * K * P
            starts = np.zeros(ngroups, dtype=np.int64)
            starts[1:] = np.cumsum(cnt)[:-1]
            within = np.arange(len(g)) - starts[g]
            slot = g * (K * P) + within
            idx = np.zeros(E_s, dtype=np.int64)
            ewp = np.zeros(E_s, dtype=np.float32)
            ldst = np.zeros(E_s, dtype=np.float32)
            idx[slot] = s_h - (half if h else 0)
            ewp[slot] = w_h
            ldst[slot] = (d_h % GROUP).astype(np.float32)
            T = E_s // P
            sfx = "hi" if h else "lo"
            out[f"eidx_{sfx}"] = _wrap_idx16(idx, E_s)
            out[f"ew_{sfx}"] = _wrap_tile_major(ewp, T)
            out[f"ldst_{sfx}"] = _wrap_tile_major(ldst, T)
        maps.append(out)
    return maps, (K_lo, K_hi)


def build_nc(n_nodes, npc, K_lh, ln_identity, tiles_per_call):
    """Build the SPMD Bass program (same program all 8 cores)."""
    K_lo, K_hi = K_lh
    ngroups = npc // GROUP
    ntile_node = npc // P  # node tiles per core
    T_lo = ngroups * K_lo  # lo-stream edge tiles per layer
    T_hi = ngroups * K_hi
    # SWDGE descriptor ring carveout is 1024 descs — one dma_gather must
    # not exceed 1024 indices (HW crashes beyond that; verified on HW).
    assert tiles_per_call * P <= 1024, tiles_per_call

    nc = bacc.Bacc(None, target_bir_lowering=False)

    # ---- I/O ----
    x_d = nc.declare_dram_parameter("x", [npc, F_IN], F32, isOutput=False)
    eidx_d, ew_d, ldst_d = {}, {}, {}
    for sfx, T in (("lo", T_lo), ("hi", T_hi)):
        eidx_d[sfx] = nc.declare_dram_parameter(
            f"eidx_{sfx}", [P, T * P // 16], I16, isOutput=False
        )
        ew_d[sfx] = nc.declare_dram_parameter(
            f"ew_{sfx}", [P, T], F32, isOutput=False
        )
        ldst_d[sfx] = nc.declare_dram_parameter(
            f"ldst_{sfx}", [P, T], F32, isOutput=False
        )
    w1_d = nc.declare_dram_parameter("w1", [F_IN, H], F32, isOutput=False)
    b1_d = nc.declare_dram_parameter("b1r", [P, H], F32, isOutput=False)
    cw_d = [
        nc.declare_dram_parameter(f"cw{i}", [H, H], F32, isOutput=False)
        for i in range(L)
    ]
    cb_d = [
        nc.declare_dram_parameter(f"cb{i}r", [P, H], F32, isOutput=False)
        for i in range(L)
    ]
    w3_d = nc.declare_dram_parameter("w3", [H, H], F32, isOutput=False)
    b3_d = nc.declare_dram_parameter("b3r", [P, H], F32, isOutput=False)
    w4_d = nc.declare_dram_parameter("w4", [H, OUT], F32, isOutput=False)
    b4_d = nc.declare_dram_parameter("b4r", [P, OUT], F32, isOutput=False)
    iota_d = nc.declare_dram_parameter("iotaf", [P, P], F32, isOutput=False)
    ident_d = nc.declare_dram_parameter("ident", [P, P], F32, isOutput=False)
    ln_d = {}
    if not ln_identity:
        ln_d["ln1g"] = nc.declare_dram_parameter("ln1g", [P, F_IN], F32, False)
        ln_d["ln1b"] = nc.declare_dram_parameter("ln1b", [P, F_IN], F32, False)
        ln_d["lng"] = nc.declare_dram_parameter("lng", [P, H], F32, False)
        ln_d["lnb"] = nc.declare_dram_parameter("lnb", [P, H], F32, False)
        ln_d["ln2g"] = nc.declare_dram_parameter("ln2g", [P, H], F32, False)
        ln_d["ln2b"] = nc.declare_dram_parameter("ln2b", [P, H], F32, False)
    out_d = nc.declare_dram_parameter("out", [npc, OUT], F32, isOutput=True)

    # ---- internal DRAM ----
    h_bounce = nc.dram_tensor("h_bounce", [npc, H], F32)
    # full-h gather table [N, H]; lo/hi streams gather from each half
    h_full = nc.dram_tensor("h_full", [n_nodes, H], F32, addr_space="Shared")

    groups_all = [list(range(NC_CORES))]

    with tile.TileContext(nc) as tc:
        with (
            tc.tile_pool(name="const", bufs=1) as cpool,
            tc.tile_pool(name="gpool", bufs=3) as gpool,
            tc.tile_pool(name="edge", bufs=8) as epool,
            tc.tile_pool(name="work", bufs=6) as wpool,
            tc.tile_pool(name="stat", bufs=8) as spool,
            tc.tile_pool(name="psA", bufs=2, space="PSUM") as psA,
            tc.tile_pool(name="psB", bufs=2, space="PSUM") as psB,
            tc.tile_pool(name="psC", bufs=2, space="PSUM") as psC,
        ):
            # ---------- persistent constants ----------
            nc.gpsimd.load_library(library_config.mlp)
            iota_f = cpool.tile([P, P], F32)
            nc.sync.dma_start(out=iota_f[:], in_=iota_d[:, :])
            ident = cpool.tile([P, P], F32)
            nc.sync.dma_start(out=ident[:], in_=ident_d[:, :])

            eidx_s, ew_s, ldst_s = {}, {}, {}
            for sfx, T in (("lo", T_lo), ("hi", T_hi)):
                eidx_s[sfx] = cpool.tile([P, T * P // 16], I16, tag=f"ei{sfx}", name=f"eidx_s_{sfx}")
                nc.sync.dma_start(out=eidx_s[sfx][:], in_=eidx_d[sfx][:, :])
                ew_s[sfx] = cpool.tile([P, T], F32, tag=f"ew{sfx}", name=f"ew_s_{sfx}")
                nc.sync.dma_start(out=ew_s[sfx][:], in_=ew_d[sfx][:, :])
                ldst_s[sfx] = cpool.tile([P, T], F32, tag=f"ld{sfx}", name=f"ldst_s_{sfx}")
                nc.sync.dma_start(out=ldst_s[sfx][:], in_=ldst_d[sfx][:, :])

            w1_s = cpool.tile([F_IN, H], F32)
            nc.sync.dma_start(out=w1_s[:], in_=w1_d[:, :])
            b1_s = cpool.tile([P, H], F32)
            nc.sync.dma_start(out=b1_s[:], in_=b1_d[:, :])
            cw_s, cb_s = [], []
            for i in range(L):
                w = cpool.tile([H, H], F32, tag=f"cw{i}")
                nc.sync.dma_start(out=w[:], in_=cw_d[i][:, :])
                cw_s.append(w)
                b = cpool.tile([P, H], F32, tag=f"cb{i}")
                nc.sync.dma_start(out=b[:], in_=cb_d[i][:, :])
                cb_s.append(b)
            w3_s = cpool.tile([H, H], F32, tag="w3")
            nc.sync.dma_start(out=w3_s[:], in_=w3_d[:, :])
            b3_s = cpool.tile([P, H], F32, tag="b3")
            nc.sync.dma_start(out=b3_s[:], in_=b3_d[:, :])
            w4_s = cpool.tile([H, OUT], F32, tag="w4")
            nc.sync.dma_start(out=w4_s[:], in_=w4_d[:, :])
            b4_s = cpool.tile([P, OUT], F32, tag="b4")
            nc.sync.dma_start(out=b4_s[:], in_=b4_d[:, :])
            ln_s = {}
            for k in ln_d:
                f = F_IN if k.startswith("ln1") else H
                t_ = cpool.tile([P, f], F32, tag=k)
                nc.sync.dma_start(out=t_[:], in_=ln_d[k][:, :])
                ln_s[k] = t_

            h_stage = cpool.tile([P, ntile_node * H], F32, tag="hstage")
            out_stage = cpool.tile([P, ntile_node * OUT], F32, tag="ostage")

            # ---------- helpers ----------
            def layer_norm(dst_ap, src_ap, f, gkey=None, bkey=None):
                """dst = LN(src) along free axis of width f. src/dst [P?, f]."""
                parts = src_ap.shape[0]
                ssum = spool.tile([P, 1], F32, tag="lnsum")
                nc.vector.tensor_reduce(
                    out=ssum[:parts], in_=src_ap, axis=AX.X, op=ALU.add
                )
                # LN is scale-invariant: y = f*x - sum = f*(x - mean)
                xc = wpool.tile([P, f], F32, tag=f"lnxc{f}")
                nc.vector.tensor_scalar(
                    out=xc[:parts],
                    in0=src_ap,
                    scalar1=float(f),
                    scalar2=ssum[:parts, 0:1],
                    op0=ALU.mult,
                    op1=ALU.subtract,
                )
                sq = wpool.tile([P, f], F32, tag=f"lnsq{f}")
                nc.vector.tensor_tensor(
                    out=sq[:parts], in0=xc[:parts], in1=xc[:parts], op=ALU.mult
                )
                vsum = spool.tile([P, 1], F32, tag="lnvar")
                nc.vector.tensor_reduce(
                    out=vsum[:parts], in_=sq[:parts], axis=AX.X, op=ALU.add
                )
                # var + eps in y-units: sum(y^2)/f^3 + eps... fold the 1/f
                # of the final normalize into rstd: apply uses y * rstd
                # where rstd = 1/(f*sqrt(var+eps)) = 1/sqrt(f^2*var+f^2*eps)
                # and f^2*var = sum(y^2)/f.
                veps = spool.tile([P, 1], F32, tag="lnveps")
                nc.vector.tensor_scalar(
                    out=veps[:parts],
                    in0=vsum[:parts],
                    scalar1=1.0 / f,
                    scalar2=float(f) * float(f) * EPS,
                    op0=ALU.mult,
                    op1=ALU.add,
                )
                std = spool.tile([P, 1], F32, tag="lnstd")
                nc.scalar.sqrt(std[:parts], veps[:parts])
                rstd = spool.tile([P, 1], F32, tag="lnrstd")
                nc.vector.reciprocal(rstd[:parts], std[:parts])
                if gkey is None:
                    nc.vector.tensor_scalar(
                        out=dst_ap,
                        in0=xc[:parts],
                        scalar1=rstd[:parts, 0:1],
                        scalar2=None,
                        op0=ALU.mult,
                    )
                else:
                    nrm = wpool.tile([P, f], F32, tag=f"lnnrm{f}")
                    nc.vector.tensor_scalar(
                        out=nrm[:parts],
                        in0=xc[:parts],
                        scalar1=rstd[:parts, 0:1],
                        scalar2=None,
                        op0=ALU.mult,
                    )
                    tmp = wpool.tile([P, f], F32, tag=f"lnaf{f}")
                    nc.vector.tensor_tensor(
                        out=tmp[:parts],
                        in0=nrm[:parts],
                        in1=ln_s[gkey][:parts],
                        op=ALU.mult,
                    )
                    nc.vector.tensor_tensor(
                        out=dst_ap,
                        in0=tmp[:parts],
                        in1=ln_s[bkey][:parts],
                        op=ALU.add,
                    )

            def elu(dst_ap, src_ap, f):
                """dst = ELU(src) = (max(x,0)-1) + exp(min(x,0))."""
                parts = src_ap.shape[0]
                r1 = wpool.tile([P, f], F32, tag=f"elur{f}")
                nc.vector.tensor_scalar(
                    out=r1[:parts],
                    in0=src_ap,
                    scalar1=0.0,
                    scalar2=1.0,
                    op0=ALU.max,
                    op1=ALU.subtract,
                )
                mn = wpool.tile([P, f], F32, tag=f"elum{f}")
                nc.vector.tensor_scalar(
                    out=mn[:parts],
                    in0=src_ap,
                    scalar1=0.0,
                    scalar2=None,
                    op0=ALU.min,
                )
                ex = wpool.tile([P, f], F32, tag=f"elue{f}")
                nc.scalar.activation(ex[:parts], mn[:parts], AF.Exp)
                nc.vector.tensor_tensor(
                    out=dst_ap, in0=r1[:parts], in1=ex[:parts], op=ALU.add
                )

            # ---------- fc_first ----------
            for t in range(ntile_node):
                xt = wpool.tile([P, F_IN], F32, tag="xt")
                nc.sync.dma_start(out=xt[:], in_=x_d[t * P : (t + 1) * P, :])
                lnx = wpool.tile([P, F_IN], F32, tag="lnx")
                if ln_identity:
                    layer_norm(lnx[:], xt[:], F_IN)
                else:
                    layer_norm(lnx[:], xt[:], F_IN, "ln1g", "ln1b")
                xT_ps = psB.tile([P, P], F32, tag="trps")
                nc.tensor.transpose(out=xT_ps[:], in_=lnx[:], identity=ident[:])
                xT = wpool.tile([P, P], F32, tag="xT")
                nc.vector.tensor_copy(xT[:], xT_ps[:])
                h_ps = psC.tile([P, H], F32, tag="linps")
                nc.tensor.matmul(
                    out=h_ps[:], lhsT=xT[:], rhs=w1_s[:], start=True, stop=True
                )
                hb = wpool.tile([P, H], F32, tag="hb")
                nc.vector.tensor_tensor(
                    out=hb[:], in0=h_ps[:], in1=b1_s[:], op=ALU.add
                )
                he = wpool.tile([P, H], F32, tag="he")
                elu(he[:], hb[:], H)
                sl = slice(t * H, (t + 1) * H)
                if ln_identity:
                    layer_norm(h_stage[:, sl], he[:], H)
                else:
                    layer_norm(h_stage[:, sl], he[:], H, "lng", "lnb")

            # DRAM view of h_bounce matching h_stage layout:
            # h_bounce[n, f], n = t*128 + p  ->  [p, t, f]
            hb_v = h_bounce[:, :].rearrange("(t p) f -> p t f", p=P)

            # ---------- conv layers ----------
            for li in range(L if "convs" not in DEBUG_SKIP else 0):
                # replicate h: shard -> bounce -> AllGather -> h_full
                nc.sync.dma_start(out=hb_v, in_=h_stage[:].rearrange(
                    "p (t f) -> p t f", f=H))
                if "collective" in DEBUG_SKIP:
                    # hang-test only: copy own shard into the table slot 0
                    nc.sync.dma_start(out=h_full[:npc, :], in_=h_bounce[:, :])
                else:
                    nc.gpsimd.collective_compute(
                        "AllGather",
                        ALU.bypass,
                        replica_groups=groups_all,
                        ins=[h_bounce[:, :]],
                        outs=[h_full[:, :]],
                    )
                gbufs = {"lo": None, "hi": None}
                half_rows = n_nodes // 2
                tables = {
                    "lo": h_full[0:half_rows, :],
                    "hi": h_full[half_rows : 2 * half_rows, :],
                }
                for g in range(ngroups):
                    agg_ps = psA.tile([P, H], F32, tag="aggps")
                    for sfx, K, T in (
                        ("lo", K_lo, T_lo),
                        ("hi", K_hi, T_hi),
                    ):
                        for k in range(K):
                            t = g * K + k
                            tt = t % tiles_per_call
                            if tt == 0:
                                c0 = t
                                n_t = min(tiles_per_call, T - c0)
                                n_e = n_t * P
                                gbufs[sfx] = gpool.tile(
                                    [P, n_t, H], F32, tag=f"gbuf{sfx}",
                                    name=f"gbuf_{sfx}"
                                )
                                if "gather" in DEBUG_SKIP:
                                    nc.vector.memset(gbufs[sfx][:], 0.0)
                                else:
                                    nc.gpsimd.dma_gather(
                                        out_ap=gbufs[sfx][:],
                                        in_ap=tables[sfx],
                                        idxs_ap=eidx_s[sfx][
                                            :,
                                            c0 * (P // 16) : c0 * (P // 16)
                                            + (n_e // 16),
                                        ],
                                        num_idxs=n_e,
                                        num_idxs_reg=n_e,
                                        elem_size=H,
                                    )
                            cm = epool.tile([P, P], F32, tag="cm")
                            nc.vector.tensor_scalar(
                                out=cm[:],
                                in0=iota_f[:],
                                scalar1=ldst_s[sfx][:, t : t + 1],
                                scalar2=ew_s[sfx][:, t : t + 1],
                                op0=ALU.is_equal,
                                op1=ALU.mult,
                            )
                            nc.tensor.matmul(
                                out=agg_ps[:],
                                lhsT=cm[:],
                                rhs=gbufs[sfx][:, tt, :],
                                start=(sfx == "lo" and k == 0),
                                stop=(sfx == "hi" and k == K - 1),
                            )
                    # --- group epilogue ---
                    agg_s = wpool.tile([P, H], F32, tag="aggs")
                    nc.vector.tensor_copy(agg_s[:], agg_ps[:])
                    tr_ps = psB.tile([H, P], F32, tag="trps")
                    nc.tensor.transpose(
                        out=tr_ps[:], in_=agg_s[:], identity=ident[:]
                    )
                    aggT = wpool.tile([H, P], F32, tag="aggT")
                    nc.vector.tensor_copy(aggT[:], tr_ps[:])
                    lin_ps = psC.tile([P, H], F32, tag="linps")
                    nc.tensor.matmul(
                        out=lin_ps[:],
                        lhsT=aggT[:],
                        rhs=cw_s[li][:],
                        start=True,
                        stop=True,
                    )
                    hbt = wpool.tile([P, H], F32, tag="hb")
                    nc.vector.tensor_tensor(
                        out=hbt[:], in0=lin_ps[:], in1=cb_s[li][:], op=ALU.add
                    )
                    hn = wpool.tile([P, H], F32, tag="hn")
                    if ln_identity:
                        layer_norm(hn[:], hbt[:], H)
                    else:
                        layer_norm(hn[:], hbt[:], H, "lng", "lnb")
                    gsl = slice(g * H, (g + 1) * H)
                    # reference invariant: res_sum == h at every layer
                    # boundary, so h_new = hn + res_sum == hn + h_old.
                    nc.vector.tensor_tensor(
                        out=h_stage[:, gsl],
                        in0=hn[:],
                        in1=h_stage[:, gsl],
                        op=ALU.add,
                    )

            # ---------- fc_final ----------
            for t in range(ntile_node):
                sl = slice(t * H, (t + 1) * H)
                lnh = wpool.tile([P, H], F32, tag="lnh")
                if ln_identity:
                    layer_norm(lnh[:], h_stage[:, sl], H)
                else:
                    layer_norm(lnh[:], h_stage[:, sl], H, "ln2g", "ln2b")
                tr_ps = psB.tile([H, P], F32, tag="trps")
                nc.tensor.transpose(out=tr_ps[:], in_=lnh[:], identity=ident[:])
                lnhT = wpool.tile([H, P], F32, tag="aggT")
                nc.vector.tensor_copy(lnhT[:], tr_ps[:])
                z_ps = psC.tile([P, H], F32, tag="linps")
                nc.tensor.matmul(
                    out=z_ps[:], lhsT=lnhT[:], rhs=w3_s[:], start=True, stop=True
                )
                zb = wpool.tile([P, H], F32, tag="hb")
                nc.vector.tensor_tensor(
                    out=zb[:], in0=z_ps[:], in1=b3_s[:], op=ALU.add
                )
                ze = wpool.tile([P, H], F32, tag="he")
                elu(ze[:], zb[:], H)
                tr2_ps = psB.tile([H, P], F32, tag="trps")
                nc.tensor.transpose(out=tr2_ps[:], in_=ze[:], identity=ident[:])
                zT = wpool.tile([H, P], F32, tag="aggT")
                nc.vector.tensor_copy(zT[:], tr2_ps[:])
                o_ps = psC.tile([P, OUT], F32, tag="ops")
                nc.tensor.matmul(
                    out=o_ps[:], lhsT=zT[:], rhs=w4_s[:], start=True, stop=True
                )
                osl = slice(t * OUT, (t + 1) * OUT)
                nc.vector.tensor_tensor(
                    out=out_stage[:, osl], in0=o_ps[:], in1=b4_s[:], op=ALU.add
                )

            out_v = out_d[:, :].rearrange("(t p) f -> p t f", p=P)
            nc.sync.dma_start(
                out=out_v,
                in_=out_stage[:].rearrange("p (t f) -> p t f", f=OUT),
            )

    nc.compile()
    return nc


def _replicate(v, parts=P):
    return np.ascontiguousarray(np.tile(np.asarray(v, np.float32)[None, :], (parts, 1)))


def kernel(
    x,
    edge_weight,
    src,
    dst,
    ln1_g,
    ln1_b,
    w1,
    b1,
    ln_g,
    ln_b,
    conv_w,
    conv_b,
    ln2_g,
    ln2_b,
    w3,
    b3,
    w4,
    b4,
    _n_cores=NC_CORES,
    _tiles_per_call=None,
    _trace=False,
    _run_kwargs=None,
):
    x = np.asarray(x, np.float32)
    n_nodes = x.shape[0]
    npc = n_nodes // NC_CORES

    ln_identity = (
        np.all(ln1_g == 1) and np.all(ln1_b == 0)
        and np.all(ln_g == 1) and np.all(ln_b == 0)
        and np.all(ln2_g == 1) and np.all(ln2_b == 0)
    )

    maps, K = prep_inputs(x, np.asarray(edge_weight), np.asarray(src),
                          np.asarray(dst), n_nodes, npc)

    tiles_per_call = _tiles_per_call
    if tiles_per_call is None:
        tiles_per_call = 8  # 1024 idxs = SWDGE ring capacity

    weights = {
        "iotaf": np.tile(np.arange(P, dtype=np.float32)[None, :], (P, 1)),
        "ident": np.eye(P, dtype=np.float32),
        "w1": np.asarray(w1, np.float32),
        "b1r": _replicate(b1),
        "w3": np.asarray(w3, np.float32),
        "b3r": _replicate(b3),
        "w4": np.asarray(w4, np.float32),
        "b4r": _replicate(b4),
    }
    for i in range(L):
        weights[f"cw{i}"] = np.asarray(conv_w[i], np.float32)
        weights[f"cb{i}r"] = _replicate(conv_b[i])
    if not ln_identity:
        weights["ln1g"] = _replicate(ln1_g)
        weights["ln1b"] = _replicate(ln1_b)
        weights["lng"] = _replicate(ln_g)
        weights["lnb"] = _replicate(ln_b)
        weights["ln2g"] = _replicate(ln2_g)
        weights["ln2b"] = _replicate(ln2_b)

    in_maps = [{**m, **weights} for m in maps]

    nc = build_nc(n_nodes, npc, K, ln_identity, tiles_per_call)
    res = run_bass_kernel_spmd(
        nc, in_maps, core_ids=list(range(NC_CORES)), trace=_trace,
        **(_run_kwargs or {}),
    )
    global LAST_RESULTS
    LAST_RESULTS = res
    return np.concatenate([r["out"] for r in res.results], axis=0)


LAST_RESULTS = None

